# revision 16
# baseline (speedup 1.0000x reference)
"""ColumnParallelFusedMoeLinear grouped-GEMM kernel for 8 Trainium2 NeuronCores.

Strategy (expert/token parallel, bf16, ragged token streaming):
  Tokens are sorted by expert; m_sizes gives each expert's contiguous row
  range of x.  Core e serves expert e: it computes y_e.T = W_e @ x_e.T as a
  dense matmul with the WEIGHT tiles stationary in the PE array and the
  TOKENS as the moving/free dimension.  Because the moving dimension is
  ragged-friendly (PE cost is proportional to streamed columns, not to
  128-padded tiles), every core streams exactly m_pad = roundup(max_e m_e)
  token columns and the per-core PE cost is 16 o-tiles x 8 k-tiles x m_pad
  cycles -- the minimum for this dtype.

  All operands travel as bf16 (error ~3e-3 << 2e-2 tolerance): x.T and W.T
  are pre-cast on the host, y leaves the device as bf16 [2048, m_pad]
  (transposed) and is upcast + transposed back on the host.

  Per-core HBM traffic = x.T (~2.2 MB) + one weight (4 MB) + y.T (~4.4 MB)
  ~= 10.6 MB, well under the PE streaming time at ~360 GB/s, so the kernel
  is tensor-engine bound at ~57 us.
"""

import math

import numpy as np

_N_CORES = 8
_P = 128
_NCHUNK = 512  # PSUM bank width in fp32 = max moving free dim per matmul

# (m_pad, d_in, d_out) -> compiled program cache
_program_cache = {}


def _build_program(m_pad, d_in, d_out):
    import concourse.mybir as mybir
    import concourse.tile as tile
    from concourse import bacc

    kc_n = d_in // _P            # contraction tiles of 128 (8)
    ot_n = d_out // _P           # output-partition tiles of 128 (16)
    chunks = [(c0, min(c0 + _NCHUNK, m_pad)) for c0 in range(0, m_pad, _NCHUNK)]

    nc = bacc.Bacc("TRN2", target_bir_lowering=False, debug=False)
    xT = nc.dram_tensor("xT", [d_in, m_pad], mybir.dt.bfloat16, kind="ExternalInput")
    wT = nc.dram_tensor("wT", [d_in, d_out], mybir.dt.bfloat16, kind="ExternalInput")
    y = nc.dram_tensor("y", [d_out, m_pad], mybir.dt.bfloat16, kind="ExternalOutput")

    # partition-major 3D views: [p, kc, cols] so one DMA moves a column
    # block across ALL contraction tiles (few big strided transfers).
    xTv = xT.rearrange("(kc p) m -> p kc m", p=_P)
    wTv = wT.rearrange("(kc p) o -> p kc o", p=_P)
    y3 = y.rearrange("(ot p) m -> ot p m", p=_P)

    # w arrives in o-column blocks sized to stay just ahead of the PE's
    # ot sweep (consumption-ordered; fine-grained early, coarse later)
    w_blocks = [(0, _P), (_P, 2 * _P), (2 * _P, 4 * _P),
                (4 * _P, 8 * _P), (8 * _P, d_out)]

    with tile.TileContext(nc) as tc:
        with (
            tc.tile_pool(name="xw", bufs=1) as xwpool,
            tc.tile_pool(name="out", bufs=4) as outpool,
            tc.tile_pool(name="psum", bufs=2, space="PSUM") as psumpool,
            tc.tile_pool(name="wpsum", bufs=1, space="PSUM") as wpsumpool,
        ):
            # resident SBUF tensors, kc-major inner layout
            x_all = xwpool.tile([_P, kc_n * m_pad], mybir.dt.bfloat16, tag="x")
            w_all = xwpool.tile([_P, kc_n * d_out], mybir.dt.bfloat16, tag="w")
            scr = xwpool.tile([_P, _NCHUNK], mybir.dt.bfloat16, tag="scr")
            xv = x_all[:].rearrange("p (kc m) -> p kc m", kc=kc_n)
            wv = w_all[:].rearrange("p (kc o) -> p kc o", kc=kc_n)

            # ---- HAM warm-up: ~10 dummy matmuls on a zeroed scratch tile
            # run during the otherwise-dead DMA prologue so the PE clock
            # gate is already at 8/8 when real data lands.
            nc.gpsimd.memset(scr[:], 0.0)
            wps = wpsumpool.tile([_P, _NCHUNK], mybir.dt.float32, tag="wps")
            for i in range(10):
                nc.tensor.matmul(wps[:], scr[:, :_P], scr[:],
                                 start=True, stop=True)

            # ---- input DMA in PE consumption order; program order on one
            # HWDGE ring ~= arrival order.
            nc.sync.dma_start(xv[:, 0, :], xTv[:, 0, :])
            nc.sync.dma_start(wv[:, :, :_P], wTv[:, :, :_P])
            for kc in (1, 2):
                nc.sync.dma_start(xv[:, kc, :], xTv[:, kc, :])
            nc.sync.dma_start(wv[:, :, _P:2 * _P], wTv[:, :, _P:2 * _P])
            for kc in (3, 4):
                nc.sync.dma_start(xv[:, kc, :], xTv[:, kc, :])
            nc.sync.dma_start(wv[:, :, 2 * _P:4 * _P], wTv[:, :, 2 * _P:4 * _P])
            for kc in (5, 6, 7):
                nc.sync.dma_start(xv[:, kc, :], xTv[:, kc, :])
            for o0, o1 in w_blocks[3:]:
                nc.sync.dma_start(wv[:, :, o0:o1], wTv[:, :, o0:o1])

            # ---- compute: sweep the 16 o-tiles; for each, keep the w tile
            # stationary across all token chunks (3 matmuls per LDWEIGHTS),
            # accumulating the 8 k-tiles into per-chunk PSUM banks; then
            # drain to one bf16 row tile and store it with a single DMA.
            for ot in range(ot_n):
                pss = [psumpool.tile([_P, c1 - c0], mybir.dt.float32,
                                     tag=f"ps{ci}", name=f"ps{ci}")
                       for ci, (c0, c1) in enumerate(chunks)]
                for kc in range(kc_n):
                    for ci, (c0, c1) in enumerate(chunks):
                        nc.tensor.matmul(
                            pss[ci][:],
                            wv[:, kc, ot * _P:(ot + 1) * _P],
                            xv[:, kc, c0:c1],
                            start=(kc == 0),
                            stop=(kc == kc_n - 1),
                        )
                o = outpool.tile([_P, m_pad], mybir.dt.bfloat16, tag="o")
                # drain chunks on both engines in parallel (tail latency)
                for ci, (c0, c1) in enumerate(chunks):
                    if ci % 2 == 0:
                        nc.vector.tensor_copy(o[:, c0:c1], pss[ci][:])
                    else:
                        nc.scalar.copy(o[:, c0:c1], pss[ci][:])
                # late stores move to the sync ring (idle once inputs are
                # in) so the scalar copy+dispatch chain isn't on the tail
                seng = nc.scalar if ot < ot_n // 2 else nc.sync
                if ot == ot_n - 1 and len(chunks) > 1:
                    # tail: ship the bulk while the small last chunk's
                    # matmuls/copy are still in flight
                    cm = chunks[-1][0]
                    seng.dma_start(y3[ot][:, :cm], o[:, :cm])
                    seng.dma_start(y3[ot][:, cm:], o[:, cm:])
                else:
                    seng.dma_start(y3[ot], o[:])
    nc.compile()
    return nc


def kernel(x, weight, m_sizes):
    import ml_dtypes
    from concourse.bass_utils import run_bass_kernel_spmd

    x = np.asarray(x)
    weight = np.asarray(weight)
    m_arr = np.asarray(m_sizes, dtype=np.int64)

    T, d_in = x.shape
    E, d_out, _ = weight.shape

    off = np.cumsum(m_arr)
    starts = np.clip(np.concatenate([[0], off[:-1]]), 0, T)
    ends = np.clip(off, 0, T)
    lens = (ends - starts).astype(np.int64)

    y = np.zeros((T, d_out), dtype=np.float32)
    max_len = int(lens.max()) if len(lens) else 0
    if max_len == 0:
        return y
    m_pad = max(_P, int(math.ceil(max_len / 16)) * 16)

    key = (m_pad, d_in, d_out)
    if key not in _program_cache:
        _program_cache[key] = _build_program(m_pad, d_in, d_out)
    nc = _program_cache[key]

    bf16 = ml_dtypes.bfloat16
    in_maps = []
    for e in range(_N_CORES):
        s0, s1 = int(starts[e % E]), int(ends[e % E])
        xTe = np.zeros((d_in, m_pad), dtype=bf16)
        if s1 > s0:
            xTe[:, : s1 - s0] = x[s0:s1].T.astype(bf16)
        wTe = np.ascontiguousarray(weight[e % E].T.astype(bf16))
        in_maps.append({"xT": xTe, "wT": wTe})

    res = run_bass_kernel_spmd(nc, in_maps, core_ids=list(range(_N_CORES)))

    for e in range(E):
        s0, s1 = int(starts[e]), int(ends[e])
        if s1 > s0:
            y[s0:s1] = res.results[e]["y"][:, : s1 - s0].T.astype(np.float32)
    return y


# revision 17
# speedup vs baseline: 1.0090x; 1.0090x over previous
"""ColumnParallelFusedMoeLinear grouped-GEMM kernel for 8 Trainium2 NeuronCores.

Strategy (expert/token parallel, bf16, ragged token streaming):
  Tokens are sorted by expert; m_sizes gives each expert's contiguous row
  range of x.  Core e serves expert e: it computes y_e.T = W_e @ x_e.T as a
  dense matmul with the WEIGHT tiles stationary in the PE array and the
  TOKENS as the moving/free dimension.  Because the moving dimension is
  ragged-friendly (PE cost is proportional to streamed columns, not to
  128-padded tiles), every core streams exactly m_pad = roundup(max_e m_e)
  token columns and the per-core PE cost is 16 o-tiles x 8 k-tiles x m_pad
  cycles -- the minimum for this dtype.

  All operands travel as bf16 (error ~3e-3 << 2e-2 tolerance): x.T and W.T
  are pre-cast on the host, y leaves the device as bf16 [2048, m_pad]
  (transposed) and is upcast + transposed back on the host.

  Per-core HBM traffic = x.T (~2.2 MB) + one weight (4 MB) + y.T (~4.4 MB)
  ~= 10.6 MB, well under the PE streaming time at ~360 GB/s, so the kernel
  is tensor-engine bound at ~57 us.
"""

import math

import numpy as np

_N_CORES = 8
_P = 128
_NCHUNK = 512  # PSUM bank width in fp32 = max moving free dim per matmul

# (m_pad, d_in, d_out) -> compiled program cache
_program_cache = {}


def _build_program(m_pad, d_in, d_out):
    import concourse.mybir as mybir
    import concourse.tile as tile
    from concourse import bacc

    kc_n = d_in // _P            # contraction tiles of 128 (8)
    ot_n = d_out // _P           # output-partition tiles of 128 (16)
    chunks = [(c0, min(c0 + _NCHUNK, m_pad)) for c0 in range(0, m_pad, _NCHUNK)]

    nc = bacc.Bacc("TRN2", target_bir_lowering=False, debug=False)
    xT = nc.dram_tensor("xT", [d_in, m_pad], mybir.dt.bfloat16, kind="ExternalInput")
    wT = nc.dram_tensor("wT", [d_in, d_out], mybir.dt.bfloat16, kind="ExternalInput")
    y = nc.dram_tensor("y", [d_out, m_pad], mybir.dt.bfloat16, kind="ExternalOutput")

    # partition-major 3D views: [p, kc, cols] so one DMA moves a column
    # block across ALL contraction tiles (few big strided transfers).
    xTv = xT.rearrange("(kc p) m -> p kc m", p=_P)
    wTv = wT.rearrange("(kc p) o -> p kc o", p=_P)
    y3 = y.rearrange("(ot p) m -> ot p m", p=_P)

    # w arrives in o-column blocks sized to stay just ahead of the PE's
    # ot sweep (consumption-ordered; fine-grained early, coarse later)
    w_blocks = [(0, _P), (_P, 2 * _P), (2 * _P, 4 * _P),
                (4 * _P, 8 * _P), (8 * _P, d_out)]

    with tile.TileContext(nc) as tc:
        with (
            tc.tile_pool(name="xw", bufs=1) as xwpool,
            tc.tile_pool(name="out", bufs=4) as outpool,
            tc.tile_pool(name="psum", bufs=2, space="PSUM") as psumpool,
            tc.tile_pool(name="wpsum", bufs=1, space="PSUM") as wpsumpool,
        ):
            # resident SBUF tensors, kc-major inner layout
            x_all = xwpool.tile([_P, kc_n * m_pad], mybir.dt.bfloat16, tag="x")
            w_all = xwpool.tile([_P, kc_n * d_out], mybir.dt.bfloat16, tag="w")
            scr = xwpool.tile([_P, _NCHUNK], mybir.dt.bfloat16, tag="scr")
            xv = x_all[:].rearrange("p (kc m) -> p kc m", kc=kc_n)
            wv = w_all[:].rearrange("p (kc o) -> p kc o", kc=kc_n)

            # ---- HAM warm-up: ~10 dummy matmuls on a zeroed scratch tile
            # run during the otherwise-dead DMA prologue so the PE clock
            # gate is already at 8/8 when real data lands.
            nc.vector.memset(scr[:], 0.0)
            wps = wpsumpool.tile([_P, _NCHUNK], mybir.dt.float32, tag="wps")
            for i in range(8):
                nc.tensor.matmul(wps[:], scr[:, :_P], scr[:],
                                 start=True, stop=True)

            # ---- input DMA in PE consumption order; program order on one
            # HWDGE ring ~= arrival order.
            nc.sync.dma_start(xv[:, 0, :], xTv[:, 0, :])
            nc.sync.dma_start(wv[:, :, :_P], wTv[:, :, :_P])
            for kc in (1, 2):
                nc.sync.dma_start(xv[:, kc, :], xTv[:, kc, :])
            nc.sync.dma_start(wv[:, :, _P:2 * _P], wTv[:, :, _P:2 * _P])
            for kc in (3, 4):
                nc.sync.dma_start(xv[:, kc, :], xTv[:, kc, :])
            nc.sync.dma_start(wv[:, :, 2 * _P:4 * _P], wTv[:, :, 2 * _P:4 * _P])
            for kc in (5, 6, 7):
                nc.sync.dma_start(xv[:, kc, :], xTv[:, kc, :])
            for o0, o1 in w_blocks[3:]:
                nc.sync.dma_start(wv[:, :, o0:o1], wTv[:, :, o0:o1])

            # ---- compute: sweep the 16 o-tiles; for each, keep the w tile
            # stationary across all token chunks (3 matmuls per LDWEIGHTS),
            # accumulating the 8 k-tiles into per-chunk PSUM banks; then
            # drain to one bf16 row tile and store it with a single DMA.
            for ot in range(ot_n):
                pss = [psumpool.tile([_P, c1 - c0], mybir.dt.float32,
                                     tag=f"ps{ci}", name=f"ps{ci}")
                       for ci, (c0, c1) in enumerate(chunks)]
                for kc in range(kc_n):
                    for ci, (c0, c1) in enumerate(chunks):
                        nc.tensor.matmul(
                            pss[ci][:],
                            wv[:, kc, ot * _P:(ot + 1) * _P],
                            xv[:, kc, c0:c1],
                            start=(kc == 0),
                            stop=(kc == kc_n - 1),
                        )
                o = outpool.tile([_P, m_pad], mybir.dt.bfloat16, tag="o")
                # drain chunks on both engines in parallel (tail latency)
                for ci, (c0, c1) in enumerate(chunks):
                    if ci % 2 == 0:
                        nc.vector.tensor_copy(o[:, c0:c1], pss[ci][:])
                    else:
                        nc.scalar.copy(o[:, c0:c1], pss[ci][:])
                # late stores move to the sync ring (idle once inputs are
                # in) so the scalar copy+dispatch chain isn't on the tail
                seng = nc.scalar if ot < ot_n // 2 else nc.sync
                if ot == ot_n - 1 and len(chunks) > 1:
                    # tail: ship the bulk while the small last chunk's
                    # matmuls/copy are still in flight
                    cm = chunks[-1][0]
                    seng.dma_start(y3[ot][:, :cm], o[:, :cm])
                    seng.dma_start(y3[ot][:, cm:], o[:, cm:])
                else:
                    seng.dma_start(y3[ot], o[:])
    nc.compile()
    return nc


def kernel(x, weight, m_sizes):
    import ml_dtypes
    from concourse.bass_utils import run_bass_kernel_spmd

    x = np.asarray(x)
    weight = np.asarray(weight)
    m_arr = np.asarray(m_sizes, dtype=np.int64)

    T, d_in = x.shape
    E, d_out, _ = weight.shape

    off = np.cumsum(m_arr)
    starts = np.clip(np.concatenate([[0], off[:-1]]), 0, T)
    ends = np.clip(off, 0, T)
    lens = (ends - starts).astype(np.int64)

    y = np.zeros((T, d_out), dtype=np.float32)
    max_len = int(lens.max()) if len(lens) else 0
    if max_len == 0:
        return y
    m_pad = max(_P, int(math.ceil(max_len / 16)) * 16)

    key = (m_pad, d_in, d_out)
    if key not in _program_cache:
        _program_cache[key] = _build_program(m_pad, d_in, d_out)
    nc = _program_cache[key]

    bf16 = ml_dtypes.bfloat16
    in_maps = []
    for e in range(_N_CORES):
        s0, s1 = int(starts[e % E]), int(ends[e % E])
        xTe = np.zeros((d_in, m_pad), dtype=bf16)
        if s1 > s0:
            xTe[:, : s1 - s0] = x[s0:s1].T.astype(bf16)
        wTe = np.ascontiguousarray(weight[e % E].T.astype(bf16))
        in_maps.append({"xT": xTe, "wT": wTe})

    res = run_bass_kernel_spmd(nc, in_maps, core_ids=list(range(_N_CORES)))

    for e in range(E):
        s0, s1 = int(starts[e]), int(ends[e])
        if s1 > s0:
            y[s0:s1] = res.results[e]["y"][:, : s1 - s0].T.astype(np.float32)
    return y
